# revision 1
# baseline (speedup 1.0000x reference)
"""CapsuleLayer (dynamic routing) Trainium2 kernel — v3 (bf16 + custom DVE).

Self-contained: shards the full inputs over 8 NeuronCores (data-parallel over
batch), runs a Bass/Tile kernel per core, gathers the full output.

Shapes (full): u [256, 1152, 8] f32, W [1152, 10, 16, 8] f32 -> v [256, 10, 16].
Per core: B=32 batches, W replicated.

Math (per core, ROUTING_ITERS=3):
  u_hat[b,i,od] = sum_k W[i,od,k] * u[b,i,k]          (od = o*16+d)
  b0 = 0; for t in 0..2: c = softmax(b, o); s = sum_i c*u_hat; v = squash(s);
  if t<2: b += sum_d u_hat*v
The t=0 step has uniform c, so v0 comes from a dense (i,k)-contraction of
ut@wr (no u_hat needed); the routing loop then only runs t=1,2.

Device layouts (i = jj*16+g, jj<72, g<16; partitions in [.]):
  wr  [(g,k)=128, (jj,od)]     bf16 host-pretransposed W
  ut  [(g,k)=128, (jj,b)]      bf16 host-pretransposed u shard
  bdu [(g,k)=128, (j,b8,g')]   bf16 block-diag u, host-packed, DMA-streamed
  u_hat [(b8,g16)=128, (jj,od)] bf16, built by PE: bdu.T @ wr per 8-batch blk
  agreement: fused custom-DVE MAC-cumsum (bf16 in, fp16 out, perf modes
    declared); per-(jj,o) sums recovered as cumsum differences at 16-element
    boundaries (fp32 scan state, so the fp16 rounding stays ~1e-3 relative).
  c build: custom-DVE multiply per b' block row writes block-diag c directly.
  s matmul: lhsT = block-diag c bf16 [(b8,g16),(b8',o)], rhs = u_hat
    -> psum[(b',o), od]
"""

import os
import sys

import numpy as np

for _p in ("/opt/trn_rl_repo", "/root/.axon_site/_ro/trn_rl_repo"):
    if os.path.isdir(_p) and _p not in sys.path:
        sys.path.insert(0, _p)

import ml_dtypes

import concourse.bacc as bacc
import concourse.bass as bass
import concourse.mybir as mybir
import concourse.tile as tile

F32 = mybir.dt.float32
F16 = mybir.dt.float16
BF16 = mybir.dt.bfloat16
NPBF = ml_dtypes.bfloat16

# Problem constants (per core)
B = 32          # local batch (256 / 8 cores)
I = 1152        # in capsules
O = 10          # out capsules
D = 16          # out dim
K = 8           # in dim
JJ = 72         # i groups of 16
G = 16          # group size
OD = O * D      # 160
BB = 8          # batch block (psum/output partition packing)
NBLK = B // BB  # 4
AC = 9          # jj per agreement/scan chunk
NCH = JJ // AC  # 8 chunks


def _register_custom_ops():
    """Register two custom DVE ops, with perf-mode table slots populated so
    the engine may run them in 2x/4x modes on packed 2-byte operands:
      SCAN_MAC4_ANT: out = cumsum(in0*in1) along free dim (fp32 state)
      MUL4X_ANT:     out = in0*in1
    """
    from concourse import dve_ops as dops
    from concourse.dve_spec import AluOp, Spec, Src0, Src1, lower, scan
    from concourse.dve_uop import DveOpSpec

    def reg(name, spec):
        existing = [op for op in dops.OPS if op.name == name]
        if existing:
            return existing[0]
        shas = {}
        specs = {}
        for ver in ("v3", "v4"):
            sp = DveOpSpec(
                name=name,
                opcode=0,
                uops=lower(spec, ver=ver),
                uops_2x=lower(spec, ver=ver),
                uops_2x_2p=lower(spec, ver=ver),
                uops_4x=lower(spec, ver=ver),
                perf_max=3,
                rd1_en=True,
            )
            shas[ver] = sp.sha(ver)
            specs[ver] = sp
        op = dops.DveOp(name, spec, subdim=False, uops_sha=shas)
        dops.OPS.append(op)
        dops.CUSTOM_DVE_SPECS[name] = spec
        dops._SUB_OPCODE_FOR_NAME[name] = dops._CUSTOM_DVE_ROW_BASE + len(dops.OPS) - 1
        assert dops._SUB_OPCODE_FOR_NAME[name] < 0x20
        row = dops.get_dve_sub_opcode(name)
        for ver, sp in specs.items():
            sp.opcode = row
            dops._COMPILE_CACHE[(name, ver)] = sp
        return op

    scan_spec = Spec(
        body=scan(AluOp.ADD, Src0 * Src1),
        reference=lambda in0, in1, c0, c1, c2: np.cumsum(
            np.asarray(in0, np.float32).reshape(in0.shape[0], -1)
            * np.asarray(in1, np.float32).reshape(in1.shape[0], -1),
            axis=-1,
        ).reshape(in0.shape),
    )
    mul_spec = Spec(
        body=Src0 * Src1,
        reference=lambda in0, in1, c0, c1, c2: (
            np.asarray(in0, np.float32) * np.asarray(in1, np.float32)
        ),
    )
    return reg("SCAN_MAC4_ANT", scan_spec), reg("MUL4X_ANT", mul_spec)


_SCAN_MAC4, _MUL4X = _register_custom_ops()


def _custom(nc, op, perf_max=0, **kw):
    # perf-mode slots are declared in the table but the vectorized uop
    # programs are not authored, so keep perf_max=0 (1x) for correctness.
    bi = nc.vector._custom_dve(op, **kw)
    bi.ins.perf_max = perf_max
    return bi


def _ap(base, free_dims, extra_offset=0):
    """AP with the base's partition dim and explicit free [step, count] dims."""
    return bass.AP(
        tensor=base.tensor,
        offset=base.offset + extra_offset,
        ap=[list(base.ap[0])] + [list(d) for d in free_dims],
    )


def _pin_act_table():
    """Make every ACT function we use resolve to the one set containing all
    of them (natural_log_exp_and_others), so bacc hoists a single
    InstLoadActFuncSet instead of thrashing Exp<->Ln sets (~1.3us/load)."""
    from concourse.bacc import get_activation_tables

    tabs = get_activation_tables("gen3")
    keep = "natural_log_exp_and_others"
    if keep not in tabs:
        return
    ours = {
        mybir.ActivationFunctionType.Exp,
        mybir.ActivationFunctionType.Ln,
        mybir.ActivationFunctionType.Square,
        mybir.ActivationFunctionType.Copy,
        mybir.ActivationFunctionType.Identity,
    }
    if not ours <= tabs[keep]:
        return
    for name, s in tabs.items():
        if name != keep:
            s -= ours


def _squash(nc, pool, s_sb, p, v_out):
    """squash over d (16) per o segment. s_sb: [p, 160] f32 sbuf -> v_out."""
    sq = pool.tile([p, OD], F32, tag="sq")
    nc.scalar.square(sq, s_sb)
    nsq = pool.tile([p, O], F32, tag="nsq")
    nc.vector.reduce_sum(
        out=nsq, in_=sq[:].rearrange("p (o d) -> p o d", d=D),
        axis=mybir.AxisListType.X,
    )
    # sqrt(x) = exp(0.5*ln(x)) — keeps ACT on one table set (ln/exp)
    rt = pool.tile([p, O], F32, tag="rt")
    nc.scalar.activation(rt, nsq, mybir.ActivationFunctionType.Ln)
    nc.scalar.activation(rt, rt, mybir.ActivationFunctionType.Exp, scale=0.5)
    nc.vector.tensor_scalar_add(rt, rt, 1e-8)     # + eps
    op1 = pool.tile([p, O], F32, tag="op1")
    nc.vector.tensor_scalar_add(op1, nsq, 1.0)    # 1 + |s|^2
    nc.vector.tensor_mul(op1, op1, rt)            # (1+n)(sqrt+eps)
    rec = pool.tile([p, O], F32, tag="rec")
    nc.vector.reciprocal(rec, op1)
    nc.vector.tensor_mul(rec, rec, nsq)           # n/((1+n)(sqrt+eps))
    nc.vector.tensor_mul(
        v_out[:].rearrange("p (o d) -> p o d", d=D),
        s_sb[:].rearrange("p (o d) -> p o d", d=D),
        _ap(rec[:], [[1, O], [0, D]]),
    )
    return v_out


def build_program():
    _pin_act_table()
    nc = bacc.Bacc("TRN2")
    wr_d = nc.dram_tensor("wr", [128, JJ * OD], BF16, kind="ExternalInput")
    ut_d = nc.dram_tensor("ut", [128, JJ * B], BF16, kind="ExternalInput")
    # block-diag u, host-packed contiguous per (blk, ch): [4, 8, 128, 1152]
    bdu_d = nc.dram_tensor(
        "bdu", [NBLK * NCH * 128, AC * BB * G], BF16, kind="ExternalInput"
    )
    mb_d = nc.dram_tensor("maskb", [128, BB * O], BF16, kind="ExternalInput")
    md_d = nc.dram_tensor("maskd", [128, OD], F32, kind="ExternalInput")
    out_d = nc.dram_tensor("v_out", [B, OD], F32, kind="ExternalOutput")

    with tile.TileContext(nc) as tc:
        with (
            tc.tile_pool(name="persist", bufs=1) as persist,
            tc.tile_pool(name="uhat", bufs=2) as uhat_pool,
            tc.tile_pool(name="bdu", bufs=2) as bdu_pool,
            tc.tile_pool(name="scano", bufs=2) as scano_pool,
            tc.tile_pool(name="blog", bufs=2) as blog_pool,
            tc.tile_pool(name="cbd", bufs=2) as cbd_pool,
            tc.tile_pool(name="soft", bufs=2) as soft_pool,
            tc.tile_pool(name="small", bufs=2) as small,
            tc.tile_pool(name="pb", bufs=4, space="PSUM") as pb_pool,
            tc.tile_pool(name="ps", bufs=2, space="PSUM") as ps_pool,
            tc.tile_pool(name="ps0", bufs=1, space="PSUM") as ps0_pool,
        ):
            # ---- resident loads ----
            wr = persist.tile([128, JJ, OD], BF16)
            for ch in range(NCH):
                nc.sync.dma_start(
                    out=wr[:, ch * AC : (ch + 1) * AC, :],
                    in_=wr_d[:, ch * AC * OD : (ch + 1) * AC * OD].rearrange(
                        "p (a b) -> p a b", b=OD
                    ),
                )
            ut = persist.tile([128, JJ, B], BF16)
            nc.sync.dma_start(
                out=ut, in_=ut_d[:].rearrange("p (a b) -> p a b", b=B)
            )
            maskb = persist.tile([128, BB * O], BF16)
            nc.sync.dma_start(out=maskb, in_=mb_d[:])
            maskd = persist.tile([128, OD], F32)
            nc.sync.dma_start(out=maskd, in_=md_d[:])

            # ---- s0 = 0.1 * sum_i u_hat  (dense (i,k) contraction) ----
            s0_ps = ps0_pool.tile([B, OD], F32)
            for jj in range(JJ):
                nc.tensor.matmul(
                    s0_ps, lhsT=ut[:, jj, :], rhs=wr[:, jj, :],
                    start=(jj == 0), stop=(jj == JJ - 1),
                )
            s0_sb = small.tile([B, OD], F32, tag="s0")
            nc.scalar.activation(
                s0_sb, s0_ps, mybir.ActivationFunctionType.Copy, scale=0.1
            )
            v0 = persist.tile([B, OD], F32, tag="v0")
            _squash(nc, small, s0_sb, B, v0)  # [32, 160]
            v0bf = persist.tile([B, OD], BF16, tag="v0bf")
            nc.scalar.copy(v0bf, v0)
            # t=1 uses v0 for every block: build all vrep tiles up front so
            # they are off the per-block critical path.
            vrep1 = []
            for blk in range(NBLK):
                vb = v0bf[:]
                src = bass.AP(
                    tensor=vb.tensor,
                    offset=vb.offset + blk * BB * vb.ap[0][0],
                    ap=[[vb.ap[0][0], BB], [0, G], [1, OD]],
                )
                vrep0 = persist.tile([128, OD], BF16, tag=f"vr0_{blk}")
                nc.scalar.dma_start(out=vrep0, in_=src)
                vr = persist.tile([128, AC * OD], BF16, tag=f"vr1_{blk}")
                nc.vector.tensor_copy(vr, _ap(vrep0[:], [[0, AC], [1, OD]]))
                vrep1.append(vr)

            # ---- per 8-batch block: build u_hat then route ----
            for blk in range(NBLK):
                u_hat = uhat_pool.tile([128, JJ, OD], BF16)
                for ch in range(NCH):  # 9 jj per chunk
                    bdu = bdu_pool.tile([128, AC, BB, G], BF16)
                    nc.sync.dma_start(
                        out=bdu,
                        in_=bdu_d[
                            (blk * NCH + ch) * 128 : (blk * NCH + ch + 1) * 128, :
                        ].rearrange("p (a b g) -> p a b g", b=BB, g=G),
                    )
                    for j3 in range(3):  # 3-jj groups share one psum bank
                        ps = pb_pool.tile([128, 3, OD], F32)
                        for j in range(3):
                            jj = ch * AC + j3 * 3 + j
                            nc.tensor.matmul(
                                ps[:, j, :], lhsT=bdu[:, j3 * 3 + j, :, :],
                                rhs=wr[:, jj, :], start=True, stop=True,
                            )
                        jj0 = ch * AC + j3 * 3
                        nc.scalar.copy(u_hat[:, jj0 : jj0 + 3, :], ps)

                blog = blog_pool.tile([128, JJ, O], F32)
                vcur = None  # [80, 16] f32 tile holding v_t rows for blk
                for t in (1, 2):
                    # -- vrep: v_{t-1} replicated to [(b,g), 9*od] bf16 --
                    if t == 1:
                        vrep = vrep1[blk]
                    else:
                        vcurbf = small.tile([O * BB, D], BF16, tag="vcbf")
                        nc.scalar.copy(vcurbf, vcur)
                        vtmp = small.tile([BB, OD], BF16, tag="vtmp")
                        nc.scalar.dma_start(out=vtmp, in_=vcurbf)
                        vt = vtmp[:]
                        src = bass.AP(
                            tensor=vt.tensor, offset=vt.offset,
                            ap=[[vt.ap[0][0], BB], [0, G], [1, OD]],
                        )
                        vrep0 = small.tile([128, OD], BF16, tag="vrep0")
                        nc.scalar.dma_start(out=vrep0, in_=src)
                        vrep = small.tile([128, AC * OD], BF16, tag="vrep")
                        nc.vector.tensor_copy(
                            vrep, _ap(vrep0[:], [[0, AC], [1, OD]])
                        )

                    # -- agreement: fused MAC-cumsum per 9-jj chunk --
                    scano = scano_pool.tile([128, NCH, AC * OD], F16)
                    for h in range(NCH):
                        uh2 = u_hat[:, h * AC : (h + 1) * AC, :].rearrange(
                            "p a b -> p (a b)"
                        )
                        _custom(
                            nc, _SCAN_MAC4, perf_max=0,
                            out=scano[:, h, :], in0=uh2, in1=vrep[:],
                        )
                    # batched cumsum-difference extraction into blog (f32):
                    #   hi = scano[.., 16n+15]; blog = hi (t1) / blog+hi (t2)
                    #   blog[.., n>0] -= hi[.., n-1]
                    sv = scano[:]
                    s_hi = _ap(sv, [[AC * OD, NCH], [D, AC * O]], D - 1)
                    s_lo = _ap(sv, [[AC * OD, NCH], [D, AC * O - 1]], D - 1)
                    bl3 = _ap(blog[:], [[AC * O, NCH], [1, AC * O]])
                    bl3s = _ap(blog[:], [[AC * O, NCH], [1, AC * O - 1]], 1)
                    if t == 1:
                        nc.vector.tensor_copy(bl3, s_hi)
                    else:
                        nc.vector.tensor_add(bl3, bl3, s_hi)
                    nc.vector.tensor_sub(bl3s, bl3s, s_lo)

                    # -- c = softmax(blog) over o (unnormalized exp * 1/D) --
                    e_bf = soft_pool.tile([128, JJ, O], BF16, tag="ebf")
                    nc.scalar.activation(
                        e_bf, blog, mybir.ActivationFunctionType.Exp
                    )
                    dsum = small.tile([128, JJ], F32, tag="dsum")
                    nc.vector.reduce_sum(
                        out=dsum, in_=e_bf, axis=mybir.AxisListType.X
                    )
                    drec = small.tile([128, JJ], F32, tag="drec")
                    nc.vector.reciprocal(drec, dsum)
                    e_n = soft_pool.tile([128, JJ, O], BF16, tag="en")
                    nc.gpsimd.tensor_mul(
                        e_n, e_bf, _ap(drec[:], [[1, JJ], [0, O]])
                    )
                    # -- cbd: block-diag c, broadcast multiply split so the
                    #    idle GPSIMD engine carries 2 of the 8 b' rows --
                    cbd = cbd_pool.tile([128, JJ, BB, O], BF16)
                    nc.vector.tensor_mul(
                        cbd[:, :, 0:6, :],
                        _ap(e_n[:], [[O, JJ], [0, 6], [1, O]]),
                        _ap(maskb[:], [[0, JJ], [O, 6], [1, O]]),
                    )
                    nc.gpsimd.tensor_mul(
                        cbd[:, :, 6:8, :],
                        _ap(e_n[:], [[O, JJ], [0, 2], [1, O]]),
                        _ap(maskb[:], [[0, JJ], [O, 2], [1, O]], 6 * O),
                    )

                    # -- s matmul: lhsT = cbd[jj], rhs = u_hat[jj] --
                    s_ps = ps_pool.tile([BB * O, OD], F32)
                    for jj in range(JJ):
                        nc.tensor.matmul(
                            s_ps, lhsT=cbd[:, jj, :, :], rhs=u_hat[:, jj, :],
                            start=(jj == 0), stop=(jj == JJ - 1),
                        )

                    # -- diag extract: s80[(b,o), d] = s_ps[(b,o), o*16+d]
                    #    via constant diag mask + reduce over o' --
                    sdm = small.tile([O * BB, OD], F32, tag="sdm")
                    nc.vector.tensor_mul(sdm, s_ps, maskd[: O * BB, :])
                    s80 = small.tile([O * BB, D], F32, tag="s80")
                    nc.vector.reduce_sum(
                        out=s80,
                        in_=sdm[:].rearrange("p (o d) -> p d o", d=D),
                        axis=mybir.AxisListType.X,
                    )
                    # squash on [(b,o), d] with per-partition scalars
                    nsq = small.tile([O * BB, 1], F32, tag="nsq80")
                    sq = small.tile([O * BB, D], F32, tag="sq80")
                    nc.scalar.square(sq, s80)
                    nc.vector.reduce_sum(
                        out=nsq, in_=sq, axis=mybir.AxisListType.X
                    )
                    # squash factor ~= sqrt(nsq)/(1+nsq)  (eps negligible);
                    # sqrt via exp(0.5*ln) to stay on one ACT table set
                    rt = small.tile([O * BB, 1], F32, tag="rt80")
                    nc.scalar.activation(
                        rt, nsq, mybir.ActivationFunctionType.Ln
                    )
                    nc.scalar.activation(
                        rt, rt, mybir.ActivationFunctionType.Exp, scale=0.5
                    )
                    op1 = small.tile([O * BB, 1], F32, tag="op180")
                    nc.vector.tensor_scalar_add(op1, nsq, 1.0)
                    rec = small.tile([O * BB, 1], F32, tag="rec80")
                    nc.vector.reciprocal(rec, op1)
                    nc.vector.tensor_mul(rec, rec, rt)
                    vcur = small.tile([O * BB, D], F32, tag="vcur")
                    nc.vector.tensor_scalar_mul(vcur, s80, rec)

                # v_out[blk*8+b, o*16+d] = vcur[b*10+o, d] (same flat order)
                nc.scalar.dma_start(
                    out=out_d[blk * BB : (blk + 1) * BB, :], in_=vcur
                )
    nc.compile()
    return nc


# ---------------- host side ----------------

_NC_CACHE = None


def _get_nc():
    global _NC_CACHE
    if _NC_CACHE is None:
        _NC_CACHE = build_program()
    return _NC_CACHE


def _pack_wr(W):
    # wr[g*8+k, jj*160 + o*16 + d] = W[jj*16+g, o, d, k]
    return np.ascontiguousarray(
        W.reshape(JJ, G, O, D, K).transpose(1, 4, 0, 2, 3).reshape(128, JJ * OD)
    ).astype(NPBF)


def _pack_ut(u_loc):
    # ut[g*8+k, jj*B + b] = u_loc[b, jj*16+g, k]
    return np.ascontiguousarray(
        u_loc.reshape(B, JJ, G, K).transpose(2, 3, 1, 0).reshape(128, JJ * B)
    ).astype(NPBF)


def _masks():
    p = np.arange(128)
    mb = (np.arange(BB)[None, :] == (p // G)[:, None]).astype(np.float32)
    mb = np.repeat(mb, O, axis=1)  # [128, 80] over (b', o)
    # maskd[(b,o) p<80, o'*16+d] = (o' == o); rows >=80 zero
    md = np.zeros((128, OD), dtype=np.float32)
    po = np.arange(O * BB) % O
    for od in range(OD):
        md[: O * BB, od] = (od // D == po).astype(np.float32)
    return mb.astype(NPBF), md


def _pack_bdu(u_loc):
    # bdu[(blk,ch)*128 + g*8+k, (j, b, g')] = u_loc[blk*8+b, (ch*9+j)*16+g', k]
    #   nonzero only when g' == g; contiguous per (blk, ch) slice.
    u4 = u_loc.reshape(NBLK, BB, NCH, AC, G, K)  # (blk, b, ch, j, g, k)
    out = np.zeros((NBLK, NCH, G, K, AC, BB, G), dtype=NPBF)
    for g in range(G):
        # (blk, ch, k, j, b)
        out[:, :, g, :, :, :, g] = u4[:, :, :, :, g, :].transpose(
            0, 2, 4, 3, 1
        ).astype(NPBF)
    return np.ascontiguousarray(out.reshape(NBLK * NCH * 128, AC * BB * G))


LAST_RESULTS = None


def kernel(u, W):
    from concourse.bass_utils import run_bass_kernel_spmd

    global LAST_RESULTS
    u = np.asarray(u, dtype=np.float32)
    W = np.asarray(W, dtype=np.float32)
    nc = _get_nc()
    wr = _pack_wr(W)
    mb, md = _masks()
    in_maps = []
    for c in range(8):
        u_loc = u[c * B : (c + 1) * B]
        in_maps.append(
            {
                "wr": wr,
                "ut": _pack_ut(u_loc),
                "bdu": _pack_bdu(u_loc),
                "maskb": mb,
                "maskd": md,
            }
        )
    trace = bool(int(os.environ.get("KBENCH_TRACE", "0")))
    try:
        res = run_bass_kernel_spmd(
            nc, in_maps, core_ids=list(range(8)), trace=trace
        )
    except ModuleNotFoundError:
        # axon NTFF hook unavailable in this container; run without trace
        res = run_bass_kernel_spmd(nc, in_maps, core_ids=list(range(8)))
    LAST_RESULTS = res
    outs = [r["v_out"].reshape(B, O, D) for r in res.results]
    return np.concatenate(outs, axis=0).astype(np.float32)



# revision 14
# speedup vs baseline: 1.7008x; 1.7008x over previous
"""CapsuleLayer (dynamic routing) Trainium2 kernel — v4 (2x scan + pipelined).

Self-contained: shards the full inputs over 8 NeuronCores (data-parallel over
batch), runs a Bass/Tile kernel per core, gathers the full output.

Shapes (full): u [256, 1152, 8] f32, W [1152, 10, 16, 8] f32 -> v [256, 10, 16].
Per core: B=32 batches, W replicated.

Math (per core, ROUTING_ITERS=3):
  u_hat[b,i,od] = sum_k W[i,od,k] * u[b,i,k]          (od = o*16+d)
  b0 = 0; for t in 0..2: c = softmax(b, o); s = sum_i c*u_hat; v = squash(s);
  if t<2: b += sum_d u_hat*v
The t=0 step has uniform c, so v0 comes from a dense (i,k)-contraction of
ut@wr (no u_hat needed); the routing loop then only runs t=1,2.

v4 structure (vs v3):
  - custom scan op carries a hand-authored 2X_1PORT uop program (pair-rate
    MAC-cumsum, fp32 state; hi outputs are the exact sequential cumsum) and
    runs at perf_max=1 — half DVE time. HW-validated standalone.
  - software pipeline: all four u_hat blocks are built first (PE+ACT), then
    the routing loop runs t-outer/blk-inner so DVE always has another
    block's scans to chew on while a block waits on its softmax/matmul.
  - scan in1 is a [128,160] v-tile broadcast via a stride-0 AP (no 1440-wide
    vrep copies).
  - block-diag c (cbd) tiles are pre-zeroed once; each stage only rewrites
    the 8 nonzero (16-partition x [72,10]) diagonal blocks with cheap
    tensor_copies split across DVE/Pool/ACT.
  - dsum/sdm/s80 reductions moved to the otherwise idle Pool engine.

Device layouts (i = jj*16+g, jj<72, g<16; partitions in [.]):
  wr  [(g,k)=128, (jj,od)]     bf16 host-pretransposed W
  ut  [(g,k)=128, (jj,b)]      bf16 host-pretransposed u shard
  bdu [(g,k)=128, (j,b8,g')]   bf16 block-diag u, host-packed, DMA-streamed
  u_hat [(b8,g16)=128, (jj,od)] bf16, built by PE: bdu.T @ wr per 8-batch blk
"""

import os
import sys

import numpy as np

for _p in ("/opt/trn_rl_repo", "/root/.axon_site/_ro/trn_rl_repo"):
    if os.path.isdir(_p) and _p not in sys.path:
        sys.path.insert(0, _p)

import ml_dtypes

import concourse.bacc as bacc
import concourse.bass as bass
import concourse.mybir as mybir
import concourse.tile as tile

F32 = mybir.dt.float32
F16 = mybir.dt.float16
BF16 = mybir.dt.bfloat16
NPBF = ml_dtypes.bfloat16

# Problem constants (per core)
B = 32          # local batch (256 / 8 cores)
I = 1152        # in capsules
O = 10          # out capsules
D = 16          # out dim
K = 8           # in dim
JJ = 72         # i groups of 16
G = 16          # group size
OD = O * D      # 160
BB = 8          # batch block (psum/output partition packing)
NBLK = B // BB  # 4
AC = 9          # jj per agreement/scan chunk
NCH = JJ // AC  # 8 chunks


def _register_custom_ops():
    """Register the fused MAC-cumsum custom DVE op with a REGULAR program
    from lower() plus a hand-authored 2X_1PORT pair program:
      per pair: out_hi = acc + p_lo + p_hi  (exact sequential cumsum)
                out_lo = out_hi - p_hi      (one extra rounding; unread)
    The kernel only consumes odd (hi) positions, which follow the same
    fp32 self-accumulate recurrence as the verified 1x program."""
    from concourse import dve_ops as dops
    from concourse.dve_spec import AluOp, Spec, Src0, Src1, lower, scan
    from concourse.dve_uop import (
        DELAY_OUT,
        DveOpSpec,
        AluInp,
        DelayInp,
        InpSel,
        OutPath,
        OutSel,
        Trigger,
        UopConfig,
        UopDpConfig,
    )

    name = "SCAN_MAC4_ANT"
    existing = [op for op in dops.OPS if op.name == name]
    if existing:
        return existing[0]

    scan_spec = Spec(
        body=scan(AluOp.ADD, Src0 * Src1),
        reference=lambda in0, in1, c0, c1, c2: np.cumsum(
            np.asarray(in0, np.float32).reshape(in0.shape[0], -1)
            * np.asarray(in1, np.float32).reshape(in1.shape[0], -1),
            axis=-1,
        ).reshape(in0.shape),
    )

    def build_2x():
        """Pair-rate scan. Lanes: d0=SRC_0, d1=SRC_1, d2=SRC_0_HI,
        d3=SRC_1_HI, d4=ZERO. blk0 p_lo; blk1 p_hi (capture p_lo->d0);
        blk2 psum=p_hi+p_lo (capture p_hi->d1); blk3 acc+=psum (seed:
        acc<-ZERO via d4); blk4 out_lo=acc-p_hi (capture acc->d2);
        blk5-7 bypass. WR0_LO<-ALU_OUT(out_lo), WR0_HI<-DELAY_2(acc)."""

        def dp(seed: bool) -> list[UopDpConfig]:
            blks = [UopDpConfig() for _ in range(8)]
            b = blks[0]
            b.enable_alu(AluOp.MULTIPLY, AluInp.PREV_DELAY_0, AluInp.PREV_DELAY_1)
            b.pass_through_delay(2, 3, 4)
            b = blks[1]
            b.enable_alu(AluOp.MULTIPLY, AluInp.PREV_DELAY_2, AluInp.PREV_DELAY_3)
            b.enable_delay_from_src(DelayInp.PREV_ALU_OUT, 0)  # p_lo
            b.pass_through_delay(4)
            b = blks[2]
            b.enable_alu(AluOp.ADD, AluInp.PREV_ALU_OUT, AluInp.PREV_DELAY_0)
            b.enable_delay_from_src(DelayInp.PREV_ALU_OUT, 1)  # p_hi
            b.pass_through_delay(4)
            b = blks[3]
            if seed:
                b.enable_alu(AluOp.BYPASS, AluInp.PREV_DELAY_4, AluInp.PREV_DELAY_4)
            else:
                b.enable_alu(AluOp.ADD, AluInp.CURR_ALU_OUT, AluInp.PREV_ALU_OUT)
            b.pass_through_delay(1)
            b = blks[4]
            b.enable_alu(AluOp.SUBTRACT, AluInp.PREV_ALU_OUT, AluInp.PREV_DELAY_1)
            b.enable_delay_from_src(DelayInp.PREV_ALU_OUT, 2)  # acc
            for k in (5, 6, 7):
                b = blks[k]
                b.pass_through_alu()
                b.pass_through_delay(2)
            return blks

        def base() -> UopConfig:
            u = UopConfig()
            u.enable_input(InpSel.SRC_0, 1)
            u.enable_input(InpSel.SRC_1, 2)
            u.enable_input(InpSel.SRC_0_HI, 3)
            u.enable_input(InpSel.SRC_1_HI, 4)
            u.enable_input(InpSel.ZERO, 5)
            return u

        seed = base()
        seed.datapath_config = dp(seed=True)
        seed.trigger = (Trigger.COUNT, Trigger.NONE, Trigger.NONE)
        seed.repeat_count = 1
        seed.next_uop = (1, 0, 0)

        steady = base()
        steady.datapath_config = dp(seed=False)
        steady.trigger = (Trigger.SRC_TENSOR_DONE, Trigger.NONE, Trigger.NONE)
        steady.next_uop = (0, 0, 0)
        steady.require_inp0 = 1
        steady.require_inp1 = 1
        steady.out[OutPath.WR0_LO] = OutSel.ALU_OUT
        steady.out_enable[OutPath.WR0_LO] = 1
        steady.out[OutPath.WR0_HI] = DELAY_OUT[2]
        steady.out_enable[OutPath.WR0_HI] = 1
        return [seed, steady]

    shas = {}
    specs = {}
    for ver in ("v3", "v4"):
        sp = DveOpSpec(
            name=name,
            opcode=0,
            uops=lower(scan_spec, ver=ver),
            uops_2x=build_2x(),
            perf_max=1,
            rd1_en=True,
        )
        shas[ver] = sp.sha(ver)
        specs[ver] = sp
    op = dops.DveOp(name, scan_spec, subdim=False, uops_sha=shas)
    dops.OPS.append(op)
    dops.CUSTOM_DVE_SPECS[name] = scan_spec
    dops._SUB_OPCODE_FOR_NAME[name] = dops._CUSTOM_DVE_ROW_BASE + len(dops.OPS) - 1
    assert dops._SUB_OPCODE_FOR_NAME[name] < 0x20
    row = dops.get_dve_sub_opcode(name)
    for ver, sp in specs.items():
        sp.opcode = row
        dops._COMPILE_CACHE[(name, ver)] = sp
    return op


_SCAN_MAC4 = _register_custom_ops()


def _custom(nc, op, perf_max=0, **kw):
    bi = nc.vector._custom_dve(op, **kw)
    bi.ins.perf_max = perf_max
    return bi


def _ap(base, free_dims, extra_offset=0):
    """AP with the base's partition dim and explicit free [step, count] dims."""
    return bass.AP(
        tensor=base.tensor,
        offset=base.offset + extra_offset,
        ap=[list(base.ap[0])] + [list(d) for d in free_dims],
    )


def _pin_act_table():
    """Make every ACT function we use resolve to the one set containing all
    of them (natural_log_exp_and_others), so bacc hoists a single
    InstLoadActFuncSet instead of thrashing Exp<->Ln sets (~1.3us/load)."""
    from concourse.bacc import get_activation_tables

    tabs = get_activation_tables("gen3")
    keep = "natural_log_exp_and_others"
    if keep not in tabs:
        return
    ours = {
        mybir.ActivationFunctionType.Exp,
        mybir.ActivationFunctionType.Ln,
        mybir.ActivationFunctionType.Square,
        mybir.ActivationFunctionType.Copy,
        mybir.ActivationFunctionType.Identity,
    }
    if not ours <= tabs[keep]:
        return
    for name, s in tabs.items():
        if name != keep:
            s -= ours


def _squash(nc, pool, s_sb, p, v_out):
    """squash over d (16) per o segment. s_sb: [p, 160] f32 sbuf -> v_out."""
    sq = pool.tile([p, OD], F32, tag="sq")
    nc.scalar.square(sq, s_sb)
    nsq = pool.tile([p, O], F32, tag="nsq")
    nc.vector.reduce_sum(
        out=nsq, in_=sq[:].rearrange("p (o d) -> p o d", d=D),
        axis=mybir.AxisListType.X,
    )
    # sqrt(x) = exp(0.5*ln(x)) — keeps ACT on one table set (ln/exp)
    rt = pool.tile([p, O], F32, tag="rt")
    nc.scalar.activation(rt, nsq, mybir.ActivationFunctionType.Ln)
    nc.scalar.activation(rt, rt, mybir.ActivationFunctionType.Exp, scale=0.5)
    nc.vector.tensor_scalar_add(rt, rt, 1e-8)     # + eps
    op1 = pool.tile([p, O], F32, tag="op1")
    nc.vector.tensor_scalar_add(op1, nsq, 1.0)    # 1 + |s|^2
    nc.vector.tensor_mul(op1, op1, rt)            # (1+n)(sqrt+eps)
    rec = pool.tile([p, O], F32, tag="rec")
    nc.vector.reciprocal(rec, op1)
    nc.vector.tensor_mul(rec, rec, nsq)           # n/((1+n)(sqrt+eps))
    nc.vector.tensor_mul(
        v_out[:].rearrange("p (o d) -> p o d", d=D),
        s_sb[:].rearrange("p (o d) -> p o d", d=D),
        _ap(rec[:], [[1, O], [0, D]]),
    )
    return v_out


def build_program():
    _pin_act_table()
    nc = bacc.Bacc("TRN2")
    wr_d = nc.dram_tensor("wr", [128, JJ * OD], BF16, kind="ExternalInput")
    ut_d = nc.dram_tensor("ut", [128, JJ * B], BF16, kind="ExternalInput")
    # block-diag u, host-packed contiguous per (blk, ch): [4, 8, 128, 1152]
    bdu_d = nc.dram_tensor(
        "bdu", [NBLK * NCH * 128, AC * BB * G], BF16, kind="ExternalInput"
    )
    md_d = nc.dram_tensor("maskd", [128, OD], F32, kind="ExternalInput")
    mb_d = nc.dram_tensor("maskb", [128, BB * O], BF16, kind="ExternalInput")
    out_d = nc.dram_tensor("v_out", [B, OD], F32, kind="ExternalOutput")

    with tile.TileContext(nc) as tc:
        with (
            tc.tile_pool(name="persist", bufs=1) as persist,
            tc.tile_pool(name="uhat", bufs=1) as uhat_pool,
            tc.tile_pool(name="bdu", bufs=2) as bdu_pool,
            tc.tile_pool(name="soft", bufs=2) as soft_pool,
            tc.tile_pool(name="small", bufs=2) as small,
            tc.tile_pool(name="pb", bufs=4, space="PSUM") as pb_pool,
            tc.tile_pool(name="ps", bufs=2, space="PSUM") as ps_pool,
            tc.tile_pool(name="ps0", bufs=1, space="PSUM") as ps0_pool,
        ):
            # ---- resident loads (ut first: it gates s0) ----
            ut = persist.tile([128, JJ, B], BF16)
            nc.sync.dma_start(
                out=ut, in_=ut_d[:].rearrange("p (a b) -> p a b", b=B)
            )
            maskd = persist.tile([128, OD], F32)
            nc.sync.dma_start(out=maskd, in_=md_d[:])
            maskb = persist.tile([128, BB * O], BF16)
            nc.sync.dma_start(out=maskb, in_=mb_d[:])
            wr = persist.tile([128, JJ, OD], BF16)

            # block-diag c double buffer, rebuilt via mask-multiply each
            # stage (engines require 32-aligned partition slices, so the
            # zeroed-buffer + 16-partition copy trick is not HW-legal).
            cbds = [
                persist.tile([128, JJ, BB, O], BF16, name=f"cbd{i}")
                for i in range(2)
            ]

            blogs = [
                persist.tile([128, JJ, O], F32, name=f"blog{b}")
                for b in range(NBLK)
            ]
            scano = persist.tile([128, NCH, AC * OD], F16, tag="scano")
            vreps = [[None, None] for _ in range(NBLK)]
            uhats = [
                uhat_pool.tile([128, JJ, OD], BF16, name=f"uh{b}")
                for b in range(NBLK)
            ]
            s0_ps = ps0_pool.tile([B, OD], F32)

            def emit_s0(ch):
                # dense (i,k)-contraction chunk for the t=0 step
                for j in range(AC):
                    jj = ch * AC + j
                    nc.tensor.matmul(
                        s0_ps, lhsT=ut[:, jj, :], rhs=wr[:, jj, :],
                        start=(jj == 0), stop=(jj == JJ - 1),
                    )

            def emit_build(blk, chunks=range(NCH)):
                # u_hat[blk] via PE block-diag matmuls; psum evicted to bf16
                # SBUF on ACT (the only engine that reads PSUM + converts
                # besides the bottleneck DVE). For blk 0 the wr chunk DMAs
                # are interleaved so PE can start immediately.
                u_hat = uhats[blk]
                for ch in chunks:
                    if blk == 0:
                        nc.sync.dma_start(
                            out=wr[:, ch * AC : (ch + 1) * AC, :],
                            in_=wr_d[
                                :, ch * AC * OD : (ch + 1) * AC * OD
                            ].rearrange("p (a b) -> p a b", b=OD),
                        )
                    bdu = bdu_pool.tile([128, AC, BB, G], BF16)
                    nc.sync.dma_start(
                        out=bdu,
                        in_=bdu_d[
                            (blk * NCH + ch) * 128 : (blk * NCH + ch + 1) * 128,
                            :,
                        ].rearrange("p (a b g) -> p a b g", b=BB, g=G),
                    )
                    if blk == 0:
                        emit_s0(ch)
                    for j3 in range(3):
                        ps = pb_pool.tile([128, 3, OD], F32)
                        for j in range(3):
                            jj = ch * AC + j3 * 3 + j
                            nc.tensor.matmul(
                                ps[:, j, :], lhsT=bdu[:, j3 * 3 + j, :, :],
                                rhs=wr[:, jj, :], start=True, stop=True,
                            )
                        jj0 = ch * AC + j3 * 3
                        nc.scalar.copy(u_hat[:, jj0 : jj0 + 3, :], ps)

            def emit_v0():
                # v0 = squash(0.1 * s0); vrep tiles for t=1 (all blocks)
                s0_sb = small.tile([B, OD], F32, tag="s0")
                nc.scalar.activation(
                    s0_sb, s0_ps, mybir.ActivationFunctionType.Copy, scale=0.1
                )
                v0 = persist.tile([B, OD], F32, tag="v0")
                _squash(nc, small, s0_sb, B, v0)
                v0bf = persist.tile([B, OD], BF16, tag="v0bf")
                nc.scalar.copy(v0bf, v0)
                for b2 in range(NBLK):
                    vb = v0bf[:]
                    src = bass.AP(
                        tensor=vb.tensor,
                        offset=vb.offset + b2 * BB * vb.ap[0][0],
                        ap=[[vb.ap[0][0], BB], [0, G], [1, OD]],
                    )
                    vr = persist.tile([128, OD], BF16, name=f"vr1_{b2}")
                    nc.sync.dma_start(out=vr, in_=src)
                    vreps[b2][0] = vr

            def emit_head(t, blk):
                # agreement scans (2x custom DVE) + blog update
                u_hat = uhats[blk]
                blog = blogs[blk]
                vrep = vreps[blk][t - 1]
                vb = vrep[:]
                v_bcast = bass.AP(
                    tensor=vb.tensor, offset=vb.offset,
                    ap=[list(vb.ap[0]), [0, AC], [1, OD]],
                )
                for h in range(NCH):
                    uh2 = u_hat[:, h * AC : (h + 1) * AC, :].rearrange(
                        "p a b -> p (a b)"
                    )
                    _custom(
                        nc, _SCAN_MAC4, perf_max=1,
                        out=scano[:, h, :], in0=uh2, in1=v_bcast,
                    )
                # cumsum-difference extraction into blog (f32):
                #   hi = scano[.., 16n+15]; blog = hi (t1) / blog+hi (t2)
                #   blog[.., n>0] -= hi[.., n-1]
                sv = scano[:]
                s_hi = _ap(sv, [[AC * OD, NCH], [D, AC * O]], D - 1)
                s_lo = _ap(sv, [[AC * OD, NCH], [D, AC * O - 1]], D - 1)
                bl3 = _ap(blog[:], [[AC * O, NCH], [1, AC * O]])
                bl3s = _ap(blog[:], [[AC * O, NCH], [1, AC * O - 1]], 1)
                if t == 1:
                    # blog[.., 0] = hi[.., 0]; blog[.., n>0] = hi[n]-hi[n-1]
                    nc.vector.tensor_copy(
                        _ap(blog[:], [[AC * O, NCH]]),
                        _ap(sv, [[AC * OD, NCH]], D - 1),
                    )
                    nc.vector.tensor_sub(
                        bl3s,
                        _ap(sv, [[AC * OD, NCH], [D, AC * O - 1]], 2 * D - 1),
                        s_lo,
                    )
                else:
                    nc.vector.tensor_add(bl3, bl3, s_hi)
                    nc.vector.tensor_sub(bl3s, bl3s, s_lo)

            def emit_tail_pre(t, blk):
                # softmax + block-diag c rewrite
                blog = blogs[blk]
                stage = (t - 1) * NBLK + blk
                e_bf = soft_pool.tile([128, JJ, O], BF16, tag="ebf")
                nc.scalar.activation(
                    e_bf, blog, mybir.ActivationFunctionType.Exp
                )
                dsum = small.tile([128, JJ], F32, tag="dsum")
                nc.vector.reduce_sum(
                    out=dsum, in_=e_bf, axis=mybir.AxisListType.X
                )
                drec = small.tile([128, JJ], F32, tag="drec")
                nc.vector.reciprocal(drec, dsum)
                e_n = soft_pool.tile([128, JJ, O], BF16, tag="en")
                nc.gpsimd.tensor_mul(
                    e_n, e_bf, _ap(drec[:], [[1, JJ], [0, O]])
                )
                cbd = cbds[stage % 2]
                nc.vector.tensor_mul(
                    cbd[:, :, 0:5, :],
                    _ap(e_n[:], [[O, JJ], [0, 5], [1, O]]),
                    _ap(maskb[:], [[0, JJ], [O, 5], [1, O]]),
                )
                nc.gpsimd.tensor_mul(
                    cbd[:, :, 5:8, :],
                    _ap(e_n[:], [[O, JJ], [0, 3], [1, O]]),
                    _ap(maskb[:], [[0, JJ], [O, 3], [1, O]], 5 * O),
                )
                return cbd

            def emit_tail_mm(t, blk, cbd):
                # s matmul: lhsT = cbd[jj], rhs = u_hat[jj]
                u_hat = uhats[blk]
                s_ps = ps_pool.tile([BB * O, OD], F32)
                for jj in range(JJ):
                    nc.tensor.matmul(
                        s_ps, lhsT=cbd[:, jj, :, :], rhs=u_hat[:, jj, :],
                        start=(jj == 0), stop=(jj == JJ - 1),
                    )
                return s_ps

            def emit_tail_post(t, blk, s_ps):
                # diag extract: s80[(b,o), d] = s_ps[(b,o), o*16+d]
                # via constant diag mask (Pool) + reduce over o' (DVE)
                sdm = small.tile([O * BB, OD], F32, tag="sdm")
                nc.vector.tensor_mul(sdm, s_ps, maskd[: O * BB, :])
                s80 = small.tile([O * BB, D], F32, tag="s80")
                nc.vector.reduce_sum(
                    out=s80,
                    in_=sdm[:].rearrange("p (o d) -> p d o", d=D),
                    axis=mybir.AxisListType.X,
                )
                # squash on [(b,o), d] with per-partition scalars
                nsq = small.tile([O * BB, 1], F32, tag="nsq80")
                sq = small.tile([O * BB, D], F32, tag="sq80")
                nc.scalar.square(sq, s80)
                nc.vector.reduce_sum(
                    out=nsq, in_=sq, axis=mybir.AxisListType.X
                )
                # squash factor ~= sqrt(nsq)/(1+nsq)  (eps negligible);
                # sqrt via exp(0.5*ln) to stay on one ACT table set
                rt = small.tile([O * BB, 1], F32, tag="rt80")
                nc.scalar.activation(
                    rt, nsq, mybir.ActivationFunctionType.Ln
                )
                nc.scalar.activation(
                    rt, rt, mybir.ActivationFunctionType.Exp, scale=0.5
                )
                op1 = small.tile([O * BB, 1], F32, tag="op180")
                nc.gpsimd.tensor_scalar_add(op1, nsq, 1.0)
                rec = small.tile([O * BB, 1], F32, tag="rec80")
                nc.vector.reciprocal(rec, op1)
                nc.gpsimd.tensor_mul(rec, rec, rt)
                vcur = small.tile([O * BB, D], F32, tag="vcur")
                nc.vector.tensor_scalar_mul(vcur, s80, rec)

                if t == 1:
                    # vrep for t=2: [80,16] -> [8,160] -> bcast [128,160]
                    vcurbf = small.tile([O * BB, D], BF16, tag="vcbf")
                    nc.vector.tensor_copy(vcurbf, vcur)
                    vtmp = small.tile([BB, OD], BF16, tag="vtmp")
                    nc.sync.dma_start(out=vtmp, in_=vcurbf)
                    vt = vtmp[:]
                    src = bass.AP(
                        tensor=vt.tensor, offset=vt.offset,
                        ap=[[vt.ap[0][0], BB], [0, G], [1, OD]],
                    )
                    vr = persist.tile([128, OD], BF16, name=f"vr2_{blk}")
                    nc.sync.dma_start(out=vr, in_=src)
                    vreps[blk][1] = vr
                else:
                    # v_out[blk*8+b, o*16+d] = vcur[b*10+o, d]
                    nc.sync.dma_start(
                        out=out_d[blk * BB : (blk + 1) * BB, :], in_=vcur
                    )

            # ---- interleaved emission: builds ride inside the routing
            # pipeline; each tail is split pre/mm/post with staggered
            # deferral so exp/e_n/smm dependencies resolve during the next
            # stage's scans ----
            emit_build(0)  # includes wr DMAs and s0 chunks
            emit_v0()
            stages = [(1, 0), (1, 1), (1, 2), (1, 3),
                      (2, 0), (2, 1), (2, 2), (2, 3)]
            cbd_of = {}
            sps_of = {}
            for s, (t, blk) in enumerate(stages):
                emit_head(t, blk)
                if s >= 1:
                    cbd_of[s - 1] = emit_tail_pre(*stages[s - 1])
                if s + 1 < NBLK:
                    emit_build(s + 1, range(0, 4))
                if s >= 1:
                    sps_of[s - 1] = emit_tail_mm(*stages[s - 1], cbd_of[s - 1])
                if s + 1 < NBLK:
                    emit_build(s + 1, range(4, NCH))
                if s >= 2:
                    emit_tail_post(*stages[s - 2], sps_of[s - 2])
            cbd_of[7] = emit_tail_pre(*stages[7])
            sps_of[7] = emit_tail_mm(*stages[7], cbd_of[7])
            emit_tail_post(*stages[6], sps_of[6])
            emit_tail_post(*stages[7], sps_of[7])
    nc.compile()
    return nc


# ---------------- host side ----------------

_NC_CACHE = None


def _get_nc():
    global _NC_CACHE
    if _NC_CACHE is None:
        _NC_CACHE = build_program()
    return _NC_CACHE


def _pack_wr(W):
    # wr[g*8+k, jj*160 + o*16 + d] = W[jj*16+g, o, d, k]
    return np.ascontiguousarray(
        W.reshape(JJ, G, O, D, K).transpose(1, 4, 0, 2, 3).reshape(128, JJ * OD)
    ).astype(NPBF)


def _pack_ut(u_loc):
    # ut[g*8+k, jj*B + b] = u_loc[b, jj*16+g, k]
    return np.ascontiguousarray(
        u_loc.reshape(B, JJ, G, K).transpose(2, 3, 1, 0).reshape(128, JJ * B)
    ).astype(NPBF)


def _maskb():
    p = np.arange(128)
    mb = (np.arange(BB)[None, :] == (p // G)[:, None]).astype(np.float32)
    return np.repeat(mb, O, axis=1).astype(NPBF)  # [128, 80] over (b', o)


def _maskd():
    # maskd[(b,o) p<80, o'*16+d] = (o' == o); rows >=80 zero
    md = np.zeros((128, OD), dtype=np.float32)
    po = np.arange(O * BB) % O
    for od in range(OD):
        md[: O * BB, od] = (od // D == po).astype(np.float32)
    return md


def _pack_bdu(u_loc):
    # bdu[(blk,ch)*128 + g*8+k, (j, b, g')] = u_loc[blk*8+b, (ch*9+j)*16+g', k]
    #   nonzero only when g' == g; contiguous per (blk, ch) slice.
    u4 = u_loc.reshape(NBLK, BB, NCH, AC, G, K)  # (blk, b, ch, j, g, k)
    out = np.zeros((NBLK, NCH, G, K, AC, BB, G), dtype=NPBF)
    for g in range(G):
        # (blk, ch, k, j, b)
        out[:, :, g, :, :, :, g] = u4[:, :, :, :, g, :].transpose(
            0, 2, 4, 3, 1
        ).astype(NPBF)
    return np.ascontiguousarray(out.reshape(NBLK * NCH * 128, AC * BB * G))


LAST_RESULTS = None


def kernel(u, W):
    from concourse.bass_utils import run_bass_kernel_spmd

    global LAST_RESULTS
    u = np.asarray(u, dtype=np.float32)
    W = np.asarray(W, dtype=np.float32)
    nc = _get_nc()
    wr = _pack_wr(W)
    md = _maskd()
    mb = _maskb()
    in_maps = []
    for c in range(8):
        u_loc = u[c * B : (c + 1) * B]
        in_maps.append(
            {
                "wr": wr,
                "ut": _pack_ut(u_loc),
                "bdu": _pack_bdu(u_loc),
                "maskd": md,
                "maskb": mb,
            }
        )
    trace = bool(int(os.environ.get("KBENCH_TRACE", "0")))
    try:
        res = run_bass_kernel_spmd(
            nc, in_maps, core_ids=list(range(8)), trace=trace
        )
    except ModuleNotFoundError:
        res = run_bass_kernel_spmd(nc, in_maps, core_ids=list(range(8)))
    LAST_RESULTS = res
    outs = [r["v_out"].reshape(B, O, D) for r in res.results]
    return np.concatenate(outs, axis=0).astype(np.float32)


# revision 73
# speedup vs baseline: 1.9749x; 1.1611x over previous
"""CapsuleLayer (dynamic routing) Trainium2 kernel — v4 (2x scan + pipelined).

Self-contained: shards the full inputs over 8 NeuronCores (data-parallel over
batch), runs a Bass/Tile kernel per core, gathers the full output.

Shapes (full): u [256, 1152, 8] f32, W [1152, 10, 16, 8] f32 -> v [256, 10, 16].
Per core: B=32 batches, W replicated.

Math (per core, ROUTING_ITERS=3):
  u_hat[b,i,od] = sum_k W[i,od,k] * u[b,i,k]          (od = o*16+d)
  b0 = 0; for t in 0..2: c = softmax(b, o); s = sum_i c*u_hat; v = squash(s);
  if t<2: b += sum_d u_hat*v
The t=0 step has uniform c, so v0 comes from a dense (i,k)-contraction of
ut@wr (no u_hat needed); the routing loop then only runs t=1,2.

v5 structure (vs v3 baseline, 263.7us -> 134.4us in the cost model):
  - custom scan op carries a hand-authored 2X_1PORT uop program (pair-rate
    MAC-cumsum, fp32 state; hi outputs are the exact sequential cumsum) and
    runs at perf_max=1 — half DVE time. HW-validated standalone
    (scan2x_test.py).
  - software pipeline, t-outer/blk-inner: u_hat builds ride inside the
    routing stages; each stage's tail is split pre/mm/post and deferred so
    its exp/e_n/s-matmul dependencies resolve under the next stage's scans.
  - scan chunks are 18 jj (4 per stage); scan in1 is a [128,160] v-tile
    broadcast via a stride-0 AP (no 1440-wide vrep copies).
  - u_hat psum evictions in 6-jj groups: [128,2,512] f32 2-bank psum tiles
    (each 3-jj half bank-aligned), one strided ACT copy per group; a few
    groups evicted by DVE where it would otherwise idle.
  - softmax denominator via a Pool-side add tree; blog updates as packed
    cumsum-difference ops; square+reduce fused via scalar_tensor_tensor
    accum_out; final squash small ops spread over DVE/ACT/Pool.
  - engine-legality constraints found on HW: GPSIMD(Pool) cannot touch
    PSUM; engine partition slices must be 32-aligned; DVE tensor_tensor
    divide is not a valid ISA op.

Device layouts (i = jj*16+g, jj<72, g<16; partitions in [.]):
  wr  [(g,k)=128, (jj,od)]     bf16 host-pretransposed W
  ut  [(g,k)=128, (jj,b)]      bf16 host-pretransposed u shard
  bdu [(g,k)=128, (j,b8,g')]   bf16 block-diag u, host-packed, DMA-streamed
  u_hat [(b8,g16)=128, (jj,od)] bf16, built by PE: bdu.T @ wr per 8-batch blk
"""

import os
import sys

import numpy as np

for _p in ("/opt/trn_rl_repo", "/root/.axon_site/_ro/trn_rl_repo"):
    if os.path.isdir(_p) and _p not in sys.path:
        sys.path.insert(0, _p)

import ml_dtypes

import concourse.bacc as bacc
import concourse.bass as bass
import concourse.mybir as mybir
import concourse.tile as tile

F32 = mybir.dt.float32
F16 = mybir.dt.float16
BF16 = mybir.dt.bfloat16
NPBF = ml_dtypes.bfloat16

# Problem constants (per core)
B = 32          # local batch (256 / 8 cores)
I = 1152        # in capsules
O = 10          # out capsules
D = 16          # out dim
K = 8           # in dim
JJ = 72         # i groups of 16
G = 16          # group size
OD = O * D      # 160
BB = 8          # batch block (psum/output partition packing)
NBLK = B // BB  # 4
AC = 18         # jj per agreement/scan chunk
NCH = JJ // AC  # 4 chunks


def _register_custom_ops():
    """Register the fused MAC-cumsum custom DVE op with a REGULAR program
    from lower() plus a hand-authored 2X_1PORT pair program:
      per pair: out_hi = acc + p_lo + p_hi  (exact sequential cumsum)
                out_lo = out_hi - p_hi      (one extra rounding; unread)
    The kernel only consumes odd (hi) positions, which follow the same
    fp32 self-accumulate recurrence as the verified 1x program."""
    from concourse import dve_ops as dops
    from concourse.dve_spec import AluOp, Spec, Src0, Src1, lower, scan
    from concourse.dve_uop import (
        DELAY_OUT,
        DveOpSpec,
        AluInp,
        DelayInp,
        InpSel,
        OutPath,
        OutSel,
        Trigger,
        UopConfig,
        UopDpConfig,
    )

    name = "SCAN_MAC4_ANT"
    existing = [op for op in dops.OPS if op.name == name]
    if existing:
        return existing[0]

    scan_spec = Spec(
        body=scan(AluOp.ADD, Src0 * Src1),
        reference=lambda in0, in1, c0, c1, c2: np.cumsum(
            np.asarray(in0, np.float32).reshape(in0.shape[0], -1)
            * np.asarray(in1, np.float32).reshape(in1.shape[0], -1),
            axis=-1,
        ).reshape(in0.shape),
    )

    def build_2x():
        """Pair-rate scan. Lanes: d0=SRC_0, d1=SRC_1, d2=SRC_0_HI,
        d3=SRC_1_HI, d4=ZERO. blk0 p_lo; blk1 p_hi (capture p_lo->d0);
        blk2 psum=p_hi+p_lo (capture p_hi->d1); blk3 acc+=psum (seed:
        acc<-ZERO via d4); blk4 out_lo=acc-p_hi (capture acc->d2);
        blk5-7 bypass. WR0_LO<-ALU_OUT(out_lo), WR0_HI<-DELAY_2(acc)."""

        def dp(seed: bool) -> list[UopDpConfig]:
            blks = [UopDpConfig() for _ in range(8)]
            b = blks[0]
            b.enable_alu(AluOp.MULTIPLY, AluInp.PREV_DELAY_0, AluInp.PREV_DELAY_1)
            b.pass_through_delay(2, 3, 4)
            b = blks[1]
            b.enable_alu(AluOp.MULTIPLY, AluInp.PREV_DELAY_2, AluInp.PREV_DELAY_3)
            b.enable_delay_from_src(DelayInp.PREV_ALU_OUT, 0)  # p_lo
            b.pass_through_delay(4)
            b = blks[2]
            b.enable_alu(AluOp.ADD, AluInp.PREV_ALU_OUT, AluInp.PREV_DELAY_0)
            b.enable_delay_from_src(DelayInp.PREV_ALU_OUT, 1)  # p_hi
            b.pass_through_delay(4)
            b = blks[3]
            if seed:
                b.enable_alu(AluOp.BYPASS, AluInp.PREV_DELAY_4, AluInp.PREV_DELAY_4)
            else:
                b.enable_alu(AluOp.ADD, AluInp.CURR_ALU_OUT, AluInp.PREV_ALU_OUT)
            b.pass_through_delay(1)
            b = blks[4]
            b.enable_alu(AluOp.SUBTRACT, AluInp.PREV_ALU_OUT, AluInp.PREV_DELAY_1)
            b.enable_delay_from_src(DelayInp.PREV_ALU_OUT, 2)  # acc
            for k in (5, 6, 7):
                b = blks[k]
                b.pass_through_alu()
                b.pass_through_delay(2)
            return blks

        def base() -> UopConfig:
            u = UopConfig()
            u.enable_input(InpSel.SRC_0, 1)
            u.enable_input(InpSel.SRC_1, 2)
            u.enable_input(InpSel.SRC_0_HI, 3)
            u.enable_input(InpSel.SRC_1_HI, 4)
            u.enable_input(InpSel.ZERO, 5)
            return u

        seed = base()
        seed.datapath_config = dp(seed=True)
        seed.trigger = (Trigger.COUNT, Trigger.NONE, Trigger.NONE)
        seed.repeat_count = 1
        seed.next_uop = (1, 0, 0)

        steady = base()
        steady.datapath_config = dp(seed=False)
        steady.trigger = (Trigger.SRC_TENSOR_DONE, Trigger.NONE, Trigger.NONE)
        steady.next_uop = (0, 0, 0)
        steady.require_inp0 = 1
        steady.require_inp1 = 1
        steady.out[OutPath.WR0_LO] = OutSel.ALU_OUT
        steady.out_enable[OutPath.WR0_LO] = 1
        steady.out[OutPath.WR0_HI] = DELAY_OUT[2]
        steady.out_enable[OutPath.WR0_HI] = 1
        return [seed, steady]

    shas = {}
    specs = {}
    for ver in ("v3", "v4"):
        sp = DveOpSpec(
            name=name,
            opcode=0,
            uops=lower(scan_spec, ver=ver),
            uops_2x=build_2x(),
            perf_max=1,
            rd1_en=True,
        )
        shas[ver] = sp.sha(ver)
        specs[ver] = sp
    op = dops.DveOp(name, scan_spec, subdim=False, uops_sha=shas)
    dops.OPS.append(op)
    dops.CUSTOM_DVE_SPECS[name] = scan_spec
    dops._SUB_OPCODE_FOR_NAME[name] = dops._CUSTOM_DVE_ROW_BASE + len(dops.OPS) - 1
    assert dops._SUB_OPCODE_FOR_NAME[name] < 0x20
    row = dops.get_dve_sub_opcode(name)
    for ver, sp in specs.items():
        sp.opcode = row
        dops._COMPILE_CACHE[(name, ver)] = sp
    return op


_SCAN_MAC4 = _register_custom_ops()


def _custom(nc, op, perf_max=0, **kw):
    bi = nc.vector._custom_dve(op, **kw)
    bi.ins.perf_max = perf_max
    return bi


def _ap(base, free_dims, extra_offset=0):
    """AP with the base's partition dim and explicit free [step, count] dims."""
    return bass.AP(
        tensor=base.tensor,
        offset=base.offset + extra_offset,
        ap=[list(base.ap[0])] + [list(d) for d in free_dims],
    )


def _pin_act_table():
    """Make every ACT function we use resolve to the one set containing all
    of them (natural_log_exp_and_others), so bacc hoists a single
    InstLoadActFuncSet instead of thrashing Exp<->Ln sets (~1.3us/load)."""
    from concourse.bacc import get_activation_tables

    tabs = get_activation_tables("gen3")
    keep = "natural_log_exp_and_others"
    if keep not in tabs:
        return
    ours = {
        mybir.ActivationFunctionType.Exp,
        mybir.ActivationFunctionType.Ln,
        mybir.ActivationFunctionType.Square,
        mybir.ActivationFunctionType.Copy,
        mybir.ActivationFunctionType.Identity,
    }
    if not ours <= tabs[keep]:
        return
    for name, s in tabs.items():
        if name != keep:
            s -= ours


def _squash(nc, pool, s_sb, p, v_out):
    """squash over d (16) per o segment. s_sb: [p, 160] f32 sbuf -> v_out."""
    sq = pool.tile([p, OD], F32, tag="sq")
    nc.vector.tensor_mul(sq, s_sb, s_sb)
    nsq = pool.tile([p, O], F32, tag="nsq")
    nc.vector.reduce_sum(
        out=nsq, in_=sq[:].rearrange("p (o d) -> p o d", d=D),
        axis=mybir.AxisListType.X,
    )
    # sqrt(x) = exp(0.5*ln(x)) — keeps ACT on one table set (ln/exp)
    rt = pool.tile([p, O], F32, tag="rt")
    nc.scalar.activation(rt, nsq, mybir.ActivationFunctionType.Ln)
    nc.scalar.activation(rt, rt, mybir.ActivationFunctionType.Exp, scale=0.5)
    op1 = pool.tile([p, O], F32, tag="op1")
    nc.vector.tensor_scalar_add(op1, nsq, 1.0)    # 1 + |s|^2
    rec = pool.tile([p, O], F32, tag="rec")
    nc.vector.reciprocal(rec, op1)
    nc.vector.tensor_mul(rec, rec, rt)            # sqrt(n)/(1+n)
    nc.vector.tensor_mul(
        v_out[:].rearrange("p (o d) -> p o d", d=D),
        s_sb[:].rearrange("p (o d) -> p o d", d=D),
        _ap(rec[:], [[1, O], [0, D]]),
    )
    return v_out


def build_program():
    _pin_act_table()
    nc = bacc.Bacc("TRN2")
    wr_d = nc.dram_tensor("wr", [128, JJ * OD], BF16, kind="ExternalInput")
    ut_d = nc.dram_tensor("ut", [128, JJ * B], BF16, kind="ExternalInput")
    # block-diag u, host-packed contiguous per (blk, ch): [4, 8, 128, 1152]
    bdu_d = nc.dram_tensor(
        "bdu", [NBLK * NCH * 128, AC * BB * G], BF16, kind="ExternalInput"
    )
    md_d = nc.dram_tensor("maskd", [128, OD], F32, kind="ExternalInput")
    mb_d = nc.dram_tensor("maskb", [128, BB * O], BF16, kind="ExternalInput")
    out_d = nc.dram_tensor("v_out", [B, OD], F32, kind="ExternalOutput")

    with tile.TileContext(nc) as tc:
        with (
            tc.tile_pool(name="persist", bufs=1) as persist,
            tc.tile_pool(name="uhat", bufs=1) as uhat_pool,
            tc.tile_pool(name="bdu", bufs=2) as bdu_pool,
            tc.tile_pool(name="soft", bufs=2) as soft_pool,
            tc.tile_pool(name="small", bufs=2) as small,
            tc.tile_pool(name="pb", bufs=2, space="PSUM") as pb_pool,
            tc.tile_pool(name="ps", bufs=2, space="PSUM") as ps_pool,
            tc.tile_pool(name="ps0", bufs=1, space="PSUM") as ps0_pool,
        ):
            # ---- resident loads (ut first: it gates s0) ----
            ut = persist.tile([128, JJ, B], BF16)
            nc.sync.dma_start(
                out=ut, in_=ut_d[:].rearrange("p (a b) -> p a b", b=B)
            )
            maskd = persist.tile([128, OD], F32)
            nc.sync.dma_start(out=maskd, in_=md_d[:])
            maskb = persist.tile([128, BB * O], BF16)
            nc.sync.dma_start(out=maskb, in_=mb_d[:])
            wr = persist.tile([128, JJ, OD], BF16)

            # block-diag c double buffer, rebuilt via mask-multiply each
            # stage (engines require 32-aligned partition slices, so the
            # zeroed-buffer + 16-partition copy trick is not HW-legal).
            cbds = [
                persist.tile([128, JJ, BB, O], BF16, name=f"cbd{i}")
                for i in range(2)
            ]

            blogs = [
                persist.tile([128, JJ, O], F32, name=f"blog{b}")
                for b in range(NBLK)
            ]
            scano = persist.tile([128, NCH, AC * OD], F16, tag="scano")
            vreps = [[None, None] for _ in range(NBLK)]
            uhats = [
                uhat_pool.tile([128, JJ, OD], BF16, name=f"uh{b}")
                for b in range(NBLK)
            ]
            s0_ps = ps0_pool.tile([B, OD], F32)

            def emit_s0(ch):
                # dense (i,k)-contraction chunk for the t=0 step
                for j in range(AC):
                    jj = ch * AC + j
                    nc.tensor.matmul(
                        s0_ps, lhsT=ut[:, jj, :], rhs=wr[:, jj, :],
                        start=(jj == 0), stop=(jj == JJ - 1),
                    )

            def emit_v0():
                # v0 = squash(0.1 * s0); vrep tiles for t=1 (all blocks)
                s0_sb = small.tile([B, OD], F32, tag="s0")
                nc.vector.tensor_scalar_mul(s0_sb, s0_ps, 0.1)
                v0 = persist.tile([B, OD], F32, tag="v0")
                _squash(nc, small, s0_sb, B, v0)
                v0bf = persist.tile([B, OD], BF16, tag="v0bf")
                nc.vector.tensor_copy(v0bf, v0)
                for b2 in range(NBLK):
                    vb = v0bf[:]
                    src = bass.AP(
                        tensor=vb.tensor,
                        offset=vb.offset + b2 * BB * vb.ap[0][0],
                        ap=[[vb.ap[0][0], BB], [0, G], [1, OD]],
                    )
                    vr = persist.tile([128, OD], BF16, name=f"vr1_{b2}")
                    nc.gpsimd.dma_start(out=vr, in_=src)
                    vreps[b2][0] = vr

            def emit_build(blk, chunks=range(NCH)):
                # u_hat[blk] via PE block-diag matmuls; psum evicted to bf16
                # SBUF on ACT (the only engine that reads PSUM + converts
                # besides the bottleneck DVE). For blk 0 the wr chunk DMAs
                # are interleaved so PE can start immediately.
                u_hat = uhats[blk]
                bdus = {}

                def issue_bdu(ch):
                    bdus[ch] = bdu_pool.tile([128, AC, BB, G], BF16, name=f"bdu{blk}_{ch}", tag="bdu")
                    nc.sync.dma_start(
                        out=bdus[ch],
                        in_=bdu_d[
                            (blk * NCH + ch) * 128 : (blk * NCH + ch + 1) * 128,
                            :,
                        ].rearrange("p (a b g) -> p a b g", b=BB, g=G),
                    )

                if blk == 0 and 0 in chunks:
                    # wr chunks first (they gate s0 -> v0 -> first scans)
                    # with bdu chunk 0 slotted in before the last wr chunk;
                    # all s0 matmuls ahead of the psum-stalled build matmuls
                    # in the PE queue.
                    for wch in range(NCH):
                        nc.sync.dma_start(
                            out=wr[:, wch * AC : (wch + 1) * AC, :],
                            in_=wr_d[
                                :, wch * AC * OD : (wch + 1) * AC * OD
                            ].rearrange("p (a b) -> p a b", b=OD),
                        )
                    for sch in range(NCH):
                        emit_s0(sch)
                    emit_v0()
                for ch in chunks:
                    if ch not in bdus:
                        issue_bdu(ch)
                    bdu = bdus[ch]
                    # 6-jj groups: one 2-bank psum tile ([128, 1024] f32 with
                    # each 3-jj half bank-aligned at 512-f32), one eviction.
                    for g6 in range(AC // 6):
                        ps = pb_pool.tile([128, 2, 512], F32)
                        for j in range(6):
                            jj = ch * AC + g6 * 6 + j
                            nc.tensor.matmul(
                                ps[:, j // 3, (j % 3) * OD : (j % 3 + 1) * OD],
                                lhsT=bdu[:, g6 * 6 + j, :, :],
                                rhs=wr[:, jj, :], start=True, stop=True,
                            )
                        jj0 = ch * AC + g6 * 6
                        src_ap = _ap(ps[:], [[512, 2], [OD, 3], [1, OD]])
                        dve_evict = (
                            False if blk == 0
                            else (ch == 1 and g6 == 1)
                        )
                        if dve_evict:
                            nc.vector.tensor_copy(
                                u_hat[:, jj0 : jj0 + 6, :], src_ap
                            )
                        else:
                            nc.scalar.copy(u_hat[:, jj0 : jj0 + 6, :], src_ap)

            def emit_head(t, blk):
                # agreement scans (2x custom DVE) + blog update
                u_hat = uhats[blk]
                blog = blogs[blk]
                vrep = vreps[blk][t - 1]
                vb = vrep[:]
                v_bcast = bass.AP(
                    tensor=vb.tensor, offset=vb.offset,
                    ap=[list(vb.ap[0]), [0, AC], [1, OD]],
                )
                for h in range(NCH):
                    uh2 = u_hat[:, h * AC : (h + 1) * AC, :].rearrange(
                        "p a b -> p (a b)"
                    )
                    _custom(
                        nc, _SCAN_MAC4, perf_max=1,
                        out=scano[:, h, :], in0=uh2, in1=v_bcast,
                    )
                # cumsum-difference extraction into blog (f32):
                #   hi = scano[.., 16n+15]; blog = hi (t1) / blog+hi (t2)
                #   blog[.., n>0] -= hi[.., n-1]
                sv = scano[:]
                s_hi = _ap(sv, [[AC * OD, NCH], [D, AC * O]], D - 1)
                s_lo = _ap(sv, [[AC * OD, NCH], [D, AC * O - 1]], D - 1)
                bl3 = _ap(blog[:], [[AC * O, NCH], [1, AC * O]])
                bl3s = _ap(blog[:], [[AC * O, NCH], [1, AC * O - 1]], 1)
                if t == 1:
                    # blog[.., 0] = hi[.., 0]; blog[.., n>0] = hi[n]-hi[n-1]
                    nc.vector.tensor_copy(
                        _ap(blog[:], [[AC * O, NCH]]),
                        _ap(sv, [[AC * OD, NCH]], D - 1),
                    )
                    nc.vector.tensor_sub(
                        bl3s,
                        _ap(sv, [[AC * OD, NCH], [D, AC * O - 1]], 2 * D - 1),
                        s_lo,
                    )
                else:
                    nc.vector.tensor_add(bl3, bl3, s_hi)
                    nc.vector.tensor_sub(bl3s, bl3s, s_lo)

            def emit_tail_pre(t, blk):
                # softmax + block-diag c rewrite
                blog = blogs[blk]
                stage = (t - 1) * NBLK + blk
                e_bf = soft_pool.tile([128, JJ, O], BF16, tag="ebf")
                nc.scalar.activation(
                    e_bf, blog, mybir.ActivationFunctionType.Exp
                )
                dsum = small.tile([128, JJ], F32, tag="dsum")
                if stage < 7:
                    # dsum via a Pool-side add tree (off the DVE)
                    t5 = small.tile([128, JJ, 5], F32, tag="dt5")
                    nc.gpsimd.tensor_add(
                        t5, e_bf[:, :, 0:5], e_bf[:, :, 5:10]
                    )
                    t2 = small.tile([128, JJ, 2], F32, tag="dt2")
                    nc.gpsimd.tensor_add(
                        t2, t5[:, :, 0:2], t5[:, :, 2:4]
                    )
                    nc.gpsimd.tensor_add(
                        dsum[:].rearrange("p (a b) -> p a b", b=1),
                        t2[:, :, 0:1], t2[:, :, 1:2],
                    )
                    nc.gpsimd.tensor_add(
                        dsum[:].rearrange("p (a b) -> p a b", b=1),
                        dsum[:].rearrange("p (a b) -> p a b", b=1),
                        t5[:, :, 4:5],
                    )
                else:
                    # late t2 stages: DVE would idle waiting on the tree
                    nc.vector.reduce_sum(
                        out=dsum, in_=e_bf, axis=mybir.AxisListType.X
                    )
                drec = small.tile([128, JJ], F32, tag="drec")
                nc.vector.reciprocal(drec, dsum)
                e_n = soft_pool.tile([128, JJ, O], BF16, tag="en")
                nc.vector.tensor_mul(
                    e_n, e_bf, _ap(drec[:], [[1, JJ], [0, O]])
                )
                cbd = cbds[stage % 2]
                nc.vector.tensor_mul(
                    cbd[:, :, 0:5, :],
                    _ap(e_n[:], [[O, JJ], [0, 5], [1, O]]),
                    _ap(maskb[:], [[0, JJ], [O, 5], [1, O]]),
                )
                nc.gpsimd.tensor_mul(
                    cbd[:, :, 5:8, :],
                    _ap(e_n[:], [[O, JJ], [0, 3], [1, O]]),
                    _ap(maskb[:], [[0, JJ], [O, 3], [1, O]], 5 * O),
                )
                return cbd

            def emit_tail_mm(t, blk, cbd):
                # s matmul: lhsT = cbd[jj], rhs = u_hat[jj]
                u_hat = uhats[blk]
                s_ps = ps_pool.tile([BB * O, OD], F32)
                for jj in range(JJ):
                    nc.tensor.matmul(
                        s_ps, lhsT=cbd[:, jj, :, :], rhs=u_hat[:, jj, :],
                        start=(jj == 0), stop=(jj == JJ - 1),
                    )
                return s_ps

            def emit_tail_post(t, blk, s_ps):
                # diag extract: s80[(b,o), d] = s_ps[(b,o), o*16+d]
                # via constant diag mask (Pool) + reduce over o' (DVE)
                sdm = small.tile([O * BB, OD], F32, tag="sdm")
                nc.vector.tensor_mul(sdm, s_ps, maskd[: O * BB, :])
                s80 = small.tile([O * BB, D], F32, tag="s80")
                nc.vector.reduce_sum(
                    out=s80,
                    in_=sdm[:].rearrange("p (o d) -> p d o", d=D),
                    axis=mybir.AxisListType.X,
                )
                # squash on [(b,o), d] with per-partition scalars
                nsq = small.tile([O * BB, 1], F32, tag="nsq80")
                sq = small.tile([O * BB, D], F32, tag="sq80")
                nc.vector.scalar_tensor_tensor(
                    out=sq, in0=s80, scalar=0.0, in1=s80,
                    op0=mybir.AluOpType.bypass, op1=mybir.AluOpType.mult,
                    accum_out=nsq,
                )
                # squash factor ~= sqrt(nsq)/(1+nsq)  (eps negligible);
                # sqrt via exp(0.5*ln) to stay on one ACT table set
                rt = small.tile([O * BB, 1], F32, tag="rt80")
                nc.scalar.activation(
                    rt, nsq, mybir.ActivationFunctionType.Ln
                )
                nc.scalar.activation(
                    rt, rt, mybir.ActivationFunctionType.Exp, scale=0.5
                )
                op1 = small.tile([O * BB, 1], F32, tag="op180")
                nc.gpsimd.tensor_scalar_add(op1, nsq, 1.0)
                rec = small.tile([O * BB, 1], F32, tag="rec80")
                nc.vector.reciprocal(rec, op1)
                nc.gpsimd.tensor_mul(rec, rec, rt)
                vcur = small.tile([O * BB, D], F32, tag="vcur")
                nc.vector.tensor_scalar_mul(vcur, s80, rec)

                if t == 1:
                    # vrep for t=2: [80,16] -> [8,160] -> bcast [128,160]
                    vcurbf = small.tile([O * BB, D], BF16, tag="vcbf")
                    nc.vector.tensor_copy(vcurbf, vcur)
                    vtmp = small.tile([BB, OD], BF16, tag="vtmp")
                    nc.sync.dma_start(out=vtmp, in_=vcurbf)
                    vt = vtmp[:]
                    src = bass.AP(
                        tensor=vt.tensor, offset=vt.offset,
                        ap=[[vt.ap[0][0], BB], [0, G], [1, OD]],
                    )
                    vr = persist.tile([128, OD], BF16, name=f"vr2_{blk}")
                    nc.sync.dma_start(out=vr, in_=src)
                    vreps[blk][1] = vr
                else:
                    # v_out[blk*8+b, o*16+d] = vcur[b*10+o, d]
                    nc.sync.dma_start(
                        out=out_d[blk * BB : (blk + 1) * BB, :], in_=vcur
                    )

            # ---- interleaved emission: builds ride inside the routing
            # pipeline; each tail is split pre/mm/post with staggered
            # deferral so exp/e_n/smm dependencies resolve during the next
            # stage's scans ----
            emit_build(0)  # includes wr DMAs, s0 chunks, and v0
            stages = [(1, 0), (1, 1), (1, 2), (1, 3),
                      (2, 0), (2, 1), (2, 2), (2, 3)]
            cbd_of = {}
            sps_of = {}
            for s, (t, blk) in enumerate(stages):
                emit_head(t, blk)
                if s >= 1:
                    cbd_of[s - 1] = emit_tail_pre(*stages[s - 1])
                if s + 1 < NBLK:
                    emit_build(s + 1, range(0, 2))
                if s >= 1:
                    sps_of[s - 1] = emit_tail_mm(*stages[s - 1], cbd_of[s - 1])
                if s >= 2:
                    emit_tail_post(*stages[s - 2], sps_of[s - 2])
                if s + 1 < NBLK:
                    emit_build(s + 1, range(2, NCH))
            cbd_of[7] = emit_tail_pre(*stages[7])
            sps_of[7] = emit_tail_mm(*stages[7], cbd_of[7])
            emit_tail_post(*stages[6], sps_of[6])
            emit_tail_post(*stages[7], sps_of[7])
    nc.compile()
    return nc


# ---------------- host side ----------------

_NC_CACHE = None


def _get_nc():
    global _NC_CACHE
    if _NC_CACHE is None:
        _NC_CACHE = build_program()
    return _NC_CACHE


def _pack_wr(W):
    # wr[g*8+k, jj*160 + o*16 + d] = W[jj*16+g, o, d, k]
    return np.ascontiguousarray(
        W.reshape(JJ, G, O, D, K).transpose(1, 4, 0, 2, 3).reshape(128, JJ * OD)
    ).astype(NPBF)


def _pack_ut(u_loc):
    # ut[g*8+k, jj*B + b] = u_loc[b, jj*16+g, k]
    return np.ascontiguousarray(
        u_loc.reshape(B, JJ, G, K).transpose(2, 3, 1, 0).reshape(128, JJ * B)
    ).astype(NPBF)


def _maskb():
    p = np.arange(128)
    mb = (np.arange(BB)[None, :] == (p // G)[:, None]).astype(np.float32)
    return np.repeat(mb, O, axis=1).astype(NPBF)  # [128, 80] over (b', o)


def _maskd():
    # maskd[(b,o) p<80, o'*16+d] = (o' == o); rows >=80 zero
    md = np.zeros((128, OD), dtype=np.float32)
    po = np.arange(O * BB) % O
    for od in range(OD):
        md[: O * BB, od] = (od // D == po).astype(np.float32)
    return md


def _pack_bdu(u_loc):
    # bdu[(blk,ch)*128 + g*8+k, (j, b, g')] = u_loc[blk*8+b, (ch*9+j)*16+g', k]
    #   nonzero only when g' == g; contiguous per (blk, ch) slice.
    u4 = u_loc.reshape(NBLK, BB, NCH, AC, G, K)  # (blk, b, ch, j, g, k)
    out = np.zeros((NBLK, NCH, G, K, AC, BB, G), dtype=NPBF)
    for g in range(G):
        # (blk, ch, k, j, b)
        out[:, :, g, :, :, :, g] = u4[:, :, :, :, g, :].transpose(
            0, 2, 4, 3, 1
        ).astype(NPBF)
    return np.ascontiguousarray(out.reshape(NBLK * NCH * 128, AC * BB * G))


LAST_RESULTS = None


def kernel(u, W):
    from concourse.bass_utils import run_bass_kernel_spmd

    global LAST_RESULTS
    u = np.asarray(u, dtype=np.float32)
    W = np.asarray(W, dtype=np.float32)
    nc = _get_nc()
    wr = _pack_wr(W)
    md = _maskd()
    mb = _maskb()
    in_maps = []
    for c in range(8):
        u_loc = u[c * B : (c + 1) * B]
        in_maps.append(
            {
                "wr": wr,
                "ut": _pack_ut(u_loc),
                "bdu": _pack_bdu(u_loc),
                "maskd": md,
                "maskb": mb,
            }
        )
    trace = bool(int(os.environ.get("KBENCH_TRACE", "0")))
    try:
        res = run_bass_kernel_spmd(
            nc, in_maps, core_ids=list(range(8)), trace=trace
        )
    except ModuleNotFoundError:
        res = run_bass_kernel_spmd(nc, in_maps, core_ids=list(range(8)))
    LAST_RESULTS = res
    outs = [r["v_out"].reshape(B, O, D) for r in res.results]
    return np.concatenate(outs, axis=0).astype(np.float32)
